# revision 35
# baseline (speedup 1.0000x reference)
"""Trainium2 Bass kernel for the KBLN scoring model.

Computes, for full inputs:
    score_l = (emb_e[e1] * emb_rel[rel]) @ emb_e.T                       (B, E)
    phi     = exp(-((lit[e1][:,None,:] - lit[None,:,:]) - c)^2 / var)    (B, E, L)
    score_n = einsum('bel,bl->be', phi, nf_weights[rel])
    out     = sigmoid(score_l + score_n)

Reformulation
-------------
With alpha[b,l] = (lit[e1[b],l] - 0.5 - c[l]) / sqrt(var[l]),
     beta[e,l]  = (lit[e,l]    - 0.5)        / sqrt(var[l]),
     g[l]       = -c[l] / sqrt(var[l]):

    phi = exp(-(alpha - beta)^2)
        = exp(-alpha^2) * exp(-(beta-g)^2 + g^2) * exp(2*(alpha-g)*beta)

The cross term x = 2*(alpha-g)*beta satisfies |x| <= 1, so a degree-3
Chebyshev (near-minimax) polynomial of exp(x) is accurate to ~6e-3.  That
turns score_n into a single matmul with contraction 4*64 = 256 rows, fused
with the 200 emb dims of score_l into one matmul chain per entity slice:

    A[b,(k,l)]  = w[b,l] * exp(-alpha^2) * cheb_k * (2*(alpha-g))^k   (host)
    Bt[(k,l),e] = exp(-(beta-g)^2 + g^2) * beta^k                     (host)

The k0/k1 Chebyshev factor tiles are bf16; the small k2/k3 tiles are
plain fp8(e4m3) (same PE rate as bf16, half the bytes), and the 200 emb
contraction rows run as ONE fp8 DoubleRow matmul per output tile.
Measured rel err of the whole scheme is ~8.5e-3 against a 2e-2 tolerance.
All factor tiles are precomputed on host, so the device program is purely:
DMA-in -> 24 accumulating matmuls -> 9 sigmoids -> DMA-out, written in
raw Bass (no Tile framework) with manual semaphores: Tile's generality
costs ~6us of whole-range semaphore clears plus per-instruction clock
traffic, which this kernel doesn't need.

Performance notes (from NTFF traces):
- Inputs are packed on host so each SBUF partition is one contiguous
  multi-KB DMA descriptor, one tensor per (entity-slice, dtype), spread
  over the two HWDGE queues (sync + scalar) in the order the PE needs
  them; SWDGE (gpsimd) only carries late outputs (its transfers are slow).
- The PE p-state reaches full clock only after ~3.4us of uninterrupted
  work, so the tensor engine warms up on junk matmuls while the first
  input slices stream in.
- Both batch halves of an entity slice are processed back-to-back and the
  entity dim is cut into 4 slices so the per-slice matmul time (~1.3us)
  matches the HBM streaming cadence (~280 B/ns/core aggregate) and the PE
  never stalls mid-stream.
- The final output tile is split by columns across the two HWDGE queues
  (sync + scalar) so its sigmoid and store pipeline, shortening the tail
  before the fixed ~7us framework postamble (whole-range semaphore clear).

Sharding: entities (E=15000) split evenly across 8 cores (1875 each);
batch side replicated; outputs concatenated on host.
"""

import sys

import numpy as np

for _p in ("/opt/trn_rl_repo", "/root/.axon_site/_ro/trn_rl_repo"):
    if _p not in sys.path:
        sys.path.append(_p)

import concourse.bass as bass
import concourse.bacc as bacc
import concourse.mybir as mybir
from concourse import bass_utils

B, E, R, D, L = 256, 15000, 237, 200, 64
NCORES = 8
ES = E // NCORES          # 1875 entities per core
KT = 4                    # polynomial terms k = 0..3
F32 = mybir.dt.float32
BF16 = mybir.dt.bfloat16
F8 = mybir.dt.float8e4
# degree-3 Chebyshev monomial coefficients of e^x on [-1,1]
CHEB = (0.99457054, 0.99730766, 0.54299068, 0.1773474)
S_SLICES = [(0, 512), (512, 448), (960, 512), (1472, 403)]
NS = len(S_SLICES)
NG = 2 * NS               # matmul groups (pairs of batch halves per slice)
NWARM = 14               # PE warm-up matmuls

TRACE = False             # test.py sets True to collect an NTFF profile
LAST = None               # last BassKernelResults (for test.py)

_PROG = None              # cached Bass program


def _build_program():
    nc = bacc.Bacc("TRN2", target_bir_lowering=False, debug=False)

    AF = mybir.ActivationFunctionType
    DR = mybir.MatmulPerfMode.DoubleRow

    # per-slice packed inputs, one contiguous run per partition.  rtb{s} is
    # a byte tensor: bytes [0:2*nsz) = bf16 k0/k1 factor tile (row i*64+p'),
    # bytes [2*nsz:3*nsz) = fp8 k2/k3 factor tile (the higher Chebyshev
    # terms are small, fp8 costs ~1e-3 extra rel err and 25% fewer bytes).
    # rte{s}[p, i*nsz+n] = emb_e.T[i*100+p, n0+n] (fp8, DoubleRow pair).
    rtb_d = [
        nc.dram_tensor(f"rtb{si}", [128, 3 * nsz + (nsz % 2)], mybir.dt.uint8,
                       kind="ExternalInput")
        for si, (n0, nsz) in enumerate(S_SLICES)
    ]
    rte_d = [
        nc.dram_tensor(f"rte{si}", [100, 2 * nsz], F8, kind="ExternalInput")
        for si, (n0, nsz) in enumerate(S_SLICES)
    ]
    lhb_d = nc.dram_tensor("lhb", [128, 3 * B], mybir.dt.uint8,
                           kind="ExternalInput")
    lhe_d = nc.dram_tensor("lhe", [100, 2 * B], F8, kind="ExternalInput")
    out_d = nc.dram_tensor("out", [B, ES], BF16, kind="ExternalOutput")

    rtb = [
        nc.alloc_sbuf_tensor(f"rtb_sb{si}", [128, 3 * nsz + (nsz % 2)],
                             mybir.dt.uint8)
        for si, (n0, nsz) in enumerate(S_SLICES)
    ]
    rte = [
        nc.alloc_sbuf_tensor(f"rte_sb{si}", [100, 2 * nsz], F8)
        for si, (n0, nsz) in enumerate(S_SLICES)
    ]
    lhb = nc.alloc_sbuf_tensor("lhb_sb", [128, 3 * B], mybir.dt.uint8)
    lhe = nc.alloc_sbuf_tensor("lhe_sb", [100, 2 * B], F8)
    obs = [
        nc.alloc_sbuf_tensor(f"ob{k}", [128, S_SLICES[k // 2][1]], BF16)
        for k in range(NG)
    ]
    # never written: stale SBUF contents feed the PE warm-up matmuls
    dummy = nc.alloc_sbuf_tensor("dummy_sb", [128, 512], BF16)
    pss = [
        nc.alloc_psum_tensor(f"ps{b}", [128, 512], F32) for b in range(8)
    ]

    s_lh = nc.alloc_semaphore("s_lh")
    s_rtc = [nc.alloc_semaphore(f"s_rtc{si}") for si in range(NS)]
    s_mm = nc.alloc_semaphore("s_mm")
    s_sig = nc.alloc_semaphore("s_sig")
    s_out = nc.alloc_semaphore("s_out")

    with nc.Block("main") as blk:

        @blk.sync
        def _(eng):
            eng.dma_start(lhb[:, :], lhb_d[:, :]).then_inc(s_lh, 16)
            for t, si in (("e", 0), ("b", 1), ("e", 2), ("e", 3)):
                sb, dr = (rtb[si], rtb_d[si]) if t == "b" else (rte[si], rte_d[si])
                eng.dma_start(sb[:, :], dr[:, :]).then_inc(s_rtc[si], 16)
            # m1 outputs (odd groups) finish 2nd, 4th, ...; the last
            # (tail-critical) one is split by columns across sync+scalar
            for si, (n0, nsz) in enumerate(S_SLICES):
                eng.wait_ge(s_sig, 2 * si + 2)
                cols = np.s_[n0 : n0 + nsz // 2] if si == NS - 1 else np.s_[n0 : n0 + nsz]
                ocol = np.s_[0 : nsz // 2] if si == NS - 1 else np.s_[0:nsz]
                eng.dma_start(
                    out_d[128:256, cols], obs[2 * si + 1][:, ocol]
                ).then_inc(s_out, 16)

        @blk.scalar
        def _(eng):
            eng.dma_start(lhe[:, :], lhe_d[:, :]).then_inc(s_lh, 16)
            for t, si in (("b", 0), ("e", 1), ("b", 2), ("b", 3)):
                sb, dr = (rtb[si], rtb_d[si]) if t == "b" else (rte[si], rte_d[si])
                eng.dma_start(sb[:, :], dr[:, :]).then_inc(s_rtc[si], 16)
            for k in range(NG - 1):
                nsz = S_SLICES[k // 2][1]
                eng.wait_ge(s_mm, k + 1)
                nc.scalar.activation(
                    obs[k][:, :], pss[k % 8][:, :nsz], AF.Sigmoid
                ).then_inc(s_sig, 1)
            # final group: split sigmoid by columns; the first half's output
            # leaves on sync while scalar finishes + sends the second half
            n0, nsz = S_SLICES[NS - 1]
            h = nsz // 2
            eng.wait_ge(s_mm, NG)
            nc.scalar.activation(
                obs[NG - 1][:, :h], pss[(NG - 1) % 8][:, :h], AF.Sigmoid
            ).then_inc(s_sig, 1)
            nc.scalar.activation(
                obs[NG - 1][:, h:], pss[(NG - 1) % 8][:, h:nsz], AF.Sigmoid
            ).then_inc(s_sig, 1)
            eng.dma_start(
                out_d[128:256, n0 + h : n0 + nsz], obs[NG - 1][:, h:]
            ).then_inc(s_out, 16)

        @blk.gpsimd
        def _(eng):
            # m0 outputs (even groups) finish 1st, 3rd, ...
            for si, (n0, nsz) in enumerate(S_SLICES):
                eng.wait_ge(s_sig, 2 * si + 1)
                eng.dma_start(
                    out_d[0:128, n0 : n0 + nsz], obs[2 * si][:, :]
                ).then_inc(s_out, 16)

        @blk.tensor
        def _(eng):
            # warm-up: keep the PE busy on junk so its p-state ramps to full
            # clock while the first input slices stream in
            for _ in range(NWARM):
                nc.tensor.matmul(
                    pss[7][:, :], dummy[:, 0:128], dummy[:, :],
                    start=True, stop=True,
                )
            lhe3 = lhe[0:100, :].rearrange("p (two f) -> p two f", two=2)
            lh0 = lhb[:, 0 : 2 * B].bitcast(BF16)       # k0/k1 rows, bf16
            lh1 = lhb[:, 2 * B : 3 * B].bitcast(F8)     # k2/k3 rows, fp8
            for si, (n0, nsz) in enumerate(S_SLICES):
                eng.wait_ge(s_rtc[si], 32)
                if si == 0:
                    eng.wait_ge(s_lh, 32)
                t0b = rtb[si][:, 0 : 2 * nsz].bitcast(BF16)
                t1f = rtb[si][:, 2 * nsz : 3 * nsz].bitcast(F8)
                rte3 = rte[si][0:100, :].rearrange("p (two f) -> p two f", two=2)
                pms = [pss[(2 * si + m) % 8] for m in range(2)]
                for m in range(2):
                    nc.tensor.matmul(
                        pms[m][:, :nsz],
                        lh0[:, m * 128 : (m + 1) * 128],
                        t0b,
                        start=True,
                        stop=False,
                    )
                    nc.tensor.matmul(
                        pms[m][:, :nsz],
                        lh1[:, m * 128 : (m + 1) * 128],
                        t1f,
                        start=False,
                        stop=False,
                    )
                for m in range(2):
                    nc.tensor.matmul(
                        pms[m][:, :nsz],
                        lhe3[:, :, m * 128 : (m + 1) * 128],
                        rte3,
                        start=False,
                        stop=True,
                        perf_mode=DR,
                    ).then_inc(s_mm, 1)

    nc.compile()
    return nc


def _host_prep(emb_e, emb_rel, nf_weights, lit, c, var, e1, rel):
    import ml_dtypes

    bf = ml_dtypes.bfloat16
    f8 = ml_dtypes.float8_e4m3
    e1 = np.asarray(e1).astype(np.int64)
    rel = np.asarray(rel).astype(np.int64)
    lit64 = np.asarray(lit, np.float64)
    c64 = np.asarray(c, np.float64)
    var64 = np.asarray(var, np.float64)

    rsv = 1.0 / np.sqrt(var64)                      # (L,)
    g = -c64 * rsv

    # ---- lhs side (batch): A[b, k*64+l] (bf16) and emb rows (fp8)
    P = lit64[e1]                                   # (B, L)
    w = np.asarray(nf_weights, np.float64)[rel]     # (B, L)
    amg = (P - 0.5) * rsv                           # alpha - g
    alpha = amg + g
    u = np.exp(-(alpha**2)) * w                     # (B, L)
    t2 = 2.0 * amg
    lhsT = np.zeros((KT * L, B), np.float64)
    acc = u.copy()
    for k in range(KT):
        if k:
            acc = acc * t2
        lhsT[k * L : (k + 1) * L, :] = (CHEB[k] * acc).T
    lhb = np.zeros((128, 3 * B), np.uint8)
    lhb[:, : 2 * B] = (
        lhsT[:128].astype(bf).view(np.uint8).reshape(128, 2 * B)
    )
    lhb[:, 2 * B :] = lhsT[128:].astype(f8).view(np.uint8).reshape(128, B)
    x = np.asarray(emb_e, np.float64)[e1] * np.asarray(emb_rel, np.float64)[rel]
    lhe = np.ascontiguousarray(
        x.T.astype(f8).reshape(2, 100, B).transpose(1, 0, 2).reshape(100, 2 * B)
    )

    # ---- rhs side (entities): Bt[k*64+l, e] = V * beta^k (bf16), emb_e.T (fp8)
    beta = (lit64 - 0.5) * rsv                      # (E, L)
    V = np.exp(beta * (2.0 * g - beta))             # (E, L)
    rhs = np.empty((KT * L, E), np.float64)
    accr = V.copy()
    for k in range(KT):
        if k:
            accr = accr * beta
        rhs[k * L : (k + 1) * L, :] = accr.T
    eT8 = np.asarray(emb_e, np.float64).T.astype(f8)   # (D, E)

    in_maps = []
    for ci in range(NCORES):
        lo = ci * ES
        R0 = rhs[:128, lo : lo + ES]
        R1 = rhs[128:, lo : lo + ES]
        Ej = eT8[:, lo : lo + ES].reshape(2, 100, ES)
        m = {"lhb": lhb, "lhe": lhe}
        for si, (n0, nsz) in enumerate(S_SLICES):
            t = np.zeros((128, 3 * nsz + (nsz % 2)), np.uint8)
            t[:, : 2 * nsz] = (
                R0[:, n0 : n0 + nsz].astype(bf).view(np.uint8)
            )
            t[:, 2 * nsz : 3 * nsz] = R1[:, n0 : n0 + nsz].astype(f8).view(np.uint8)
            m[f"rtb{si}"] = t
            m[f"rte{si}"] = np.ascontiguousarray(
                Ej[:, :, n0 : n0 + nsz].transpose(1, 0, 2).reshape(100, 2 * nsz)
            )
        in_maps.append(m)
    return in_maps


def kernel(emb_e, emb_rel, nf_weights, lit, c, var, e1, rel):
    global _PROG, LAST
    if _PROG is None:
        _PROG = _build_program()
    in_maps = _host_prep(emb_e, emb_rel, nf_weights, lit, c, var, e1, rel)
    res = bass_utils.run_bass_kernel_spmd(
        _PROG, in_maps, core_ids=list(range(NCORES)), trace=TRACE
    )
    LAST = res
    return np.concatenate(
        [np.asarray(res.results[ci]["out"]).astype(np.float32) for ci in range(NCORES)],
        axis=1,
    )


# revision 36
# speedup vs baseline: 1.0020x; 1.0020x over previous
"""Trainium2 Bass kernel for the KBLN scoring model.

Computes, for full inputs:
    score_l = (emb_e[e1] * emb_rel[rel]) @ emb_e.T                       (B, E)
    phi     = exp(-((lit[e1][:,None,:] - lit[None,:,:]) - c)^2 / var)    (B, E, L)
    score_n = einsum('bel,bl->be', phi, nf_weights[rel])
    out     = sigmoid(score_l + score_n)

Reformulation
-------------
With alpha[b,l] = (lit[e1[b],l] - 0.5 - c[l]) / sqrt(var[l]),
     beta[e,l]  = (lit[e,l]    - 0.5)        / sqrt(var[l]),
     g[l]       = -c[l] / sqrt(var[l]):

    phi = exp(-(alpha - beta)^2)
        = exp(-alpha^2) * exp(-(beta-g)^2 + g^2) * exp(2*(alpha-g)*beta)

The cross term x = 2*(alpha-g)*beta satisfies |x| <= 1, so a degree-3
Chebyshev (near-minimax) polynomial of exp(x) is accurate to ~6e-3.  That
turns score_n into a single matmul with contraction 4*64 = 256 rows, fused
with the 200 emb dims of score_l into one matmul chain per entity slice:

    A[b,(k,l)]  = w[b,l] * exp(-alpha^2) * cheb_k * (2*(alpha-g))^k   (host)
    Bt[(k,l),e] = exp(-(beta-g)^2 + g^2) * beta^k                     (host)

The k0/k1 Chebyshev factor tiles are bf16; the small k2/k3 tiles are
plain fp8(e4m3) (same PE rate as bf16, half the bytes), and the 200 emb
contraction rows run as ONE fp8 DoubleRow matmul per output tile.
Measured rel err of the whole scheme is ~8.5e-3 against a 2e-2 tolerance.
All factor tiles are precomputed on host, so the device program is purely:
DMA-in -> 24 accumulating matmuls -> 9 sigmoids -> DMA-out, written in
raw Bass (no Tile framework) with manual semaphores: Tile's generality
costs ~6us of whole-range semaphore clears plus per-instruction clock
traffic, which this kernel doesn't need.

Performance notes (from NTFF traces):
- Inputs are packed on host so each SBUF partition is one contiguous
  multi-KB DMA descriptor, one tensor per (entity-slice, dtype), spread
  over the two HWDGE queues (sync + scalar) in the order the PE needs
  them; SWDGE (gpsimd) only carries late outputs (its transfers are slow).
- The PE p-state reaches full clock only after ~3.4us of uninterrupted
  work, so the tensor engine warms up on junk matmuls while the first
  input slices stream in.
- Both batch halves of an entity slice are processed back-to-back and the
  entity dim is cut into 4 slices so the per-slice matmul time (~1.3us)
  matches the HBM streaming cadence (~280 B/ns/core aggregate) and the PE
  never stalls mid-stream.
- The final output tile is split by columns across the two HWDGE queues
  (sync + scalar) so its sigmoid and store pipeline, shortening the tail
  before the fixed ~7us framework postamble (whole-range semaphore clear).

Sharding: entities (E=15000) split evenly across 8 cores (1875 each);
batch side replicated; outputs concatenated on host.
"""

import sys

import numpy as np

for _p in ("/opt/trn_rl_repo", "/root/.axon_site/_ro/trn_rl_repo"):
    if _p not in sys.path:
        sys.path.append(_p)

import concourse.bass as bass
import concourse.bacc as bacc
import concourse.mybir as mybir
from concourse import bass_utils

B, E, R, D, L = 256, 15000, 237, 200, 64
NCORES = 8
ES = E // NCORES          # 1875 entities per core
KT = 4                    # polynomial terms k = 0..3
F32 = mybir.dt.float32
BF16 = mybir.dt.bfloat16
F8 = mybir.dt.float8e4
# degree-3 Chebyshev monomial coefficients of e^x on [-1,1]
CHEB = (0.99457054, 0.99730766, 0.54299068, 0.1773474)
S_SLICES = [(0, 512), (512, 448), (960, 512), (1472, 403)]
NS = len(S_SLICES)
NG = 2 * NS               # matmul groups (pairs of batch halves per slice)
NWARM = 14               # PE warm-up matmuls

TRACE = False             # test.py sets True to collect an NTFF profile
LAST = None               # last BassKernelResults (for test.py)

_PROG = None              # cached Bass program


def _build_program():
    nc = bacc.Bacc("TRN2", target_bir_lowering=False, debug=False)

    AF = mybir.ActivationFunctionType
    DR = mybir.MatmulPerfMode.DoubleRow

    # per-slice packed inputs, one contiguous run per partition.  rtb{s} is
    # a byte tensor: bytes [0:2*nsz) = bf16 k0/k1 factor tile (row i*64+p'),
    # bytes [2*nsz:3*nsz) = fp8 k2/k3 factor tile (the higher Chebyshev
    # terms are small, fp8 costs ~1e-3 extra rel err and 25% fewer bytes).
    # rte{s}[p, i*nsz+n] = emb_e.T[i*100+p, n0+n] (fp8, DoubleRow pair).
    rtb_d = [
        nc.dram_tensor(f"rtb{si}", [128, 3 * nsz + (nsz % 2)], mybir.dt.uint8,
                       kind="ExternalInput")
        for si, (n0, nsz) in enumerate(S_SLICES)
    ]
    rte_d = [
        nc.dram_tensor(f"rte{si}", [100, 2 * nsz], F8, kind="ExternalInput")
        for si, (n0, nsz) in enumerate(S_SLICES)
    ]
    lhb_d = nc.dram_tensor("lhb", [128, 3 * B], mybir.dt.uint8,
                           kind="ExternalInput")
    lhe_d = nc.dram_tensor("lhe", [100, 2 * B], F8, kind="ExternalInput")
    out_d = nc.dram_tensor("out", [B, ES], BF16, kind="ExternalOutput")

    rtb = [
        nc.alloc_sbuf_tensor(f"rtb_sb{si}", [128, 3 * nsz + (nsz % 2)],
                             mybir.dt.uint8)
        for si, (n0, nsz) in enumerate(S_SLICES)
    ]
    rte = [
        nc.alloc_sbuf_tensor(f"rte_sb{si}", [100, 2 * nsz], F8)
        for si, (n0, nsz) in enumerate(S_SLICES)
    ]
    lhb = nc.alloc_sbuf_tensor("lhb_sb", [128, 3 * B], mybir.dt.uint8)
    lhe = nc.alloc_sbuf_tensor("lhe_sb", [100, 2 * B], F8)
    obs = [
        nc.alloc_sbuf_tensor(f"ob{k}", [128, S_SLICES[k // 2][1]], BF16)
        for k in range(NG)
    ]
    # never written: stale SBUF contents feed the PE warm-up matmuls
    dummy = nc.alloc_sbuf_tensor("dummy_sb", [128, 512], BF16)
    pss = [
        nc.alloc_psum_tensor(f"ps{b}", [128, 512], F32) for b in range(8)
    ]

    s_lh = nc.alloc_semaphore("s_lh")
    s_rtc = [nc.alloc_semaphore(f"s_rtc{si}") for si in range(NS)]
    s_mm = nc.alloc_semaphore("s_mm")
    s_sig = nc.alloc_semaphore("s_sig")
    s_out = nc.alloc_semaphore("s_out")

    with nc.Block("main") as blk:

        @blk.sync
        def _(eng):
            eng.dma_start(lhb[:, :], lhb_d[:, :]).then_inc(s_lh, 16)
            for t, si in (("e", 0), ("b", 1), ("e", 2), ("e", 3)):
                sb, dr = (rtb[si], rtb_d[si]) if t == "b" else (rte[si], rte_d[si])
                eng.dma_start(sb[:, :], dr[:, :]).then_inc(s_rtc[si], 16)
            # m1 outputs (odd groups) finish 2nd, 4th, ...; the last
            # (tail-critical) one is split by columns across sync+scalar
            for si, (n0, nsz) in enumerate(S_SLICES):
                eng.wait_ge(s_sig, 2 * si + 2)
                cols = np.s_[n0 : n0 + nsz // 2] if si == NS - 1 else np.s_[n0 : n0 + nsz]
                ocol = np.s_[0 : nsz // 2] if si == NS - 1 else np.s_[0:nsz]
                eng.dma_start(
                    out_d[128:256, cols], obs[2 * si + 1][:, ocol]
                ).then_inc(s_out, 16)

        @blk.scalar
        def _(eng):
            eng.dma_start(lhe[:, :], lhe_d[:, :]).then_inc(s_lh, 16)
            for t, si in (("b", 0), ("e", 1), ("b", 2), ("b", 3)):
                sb, dr = (rtb[si], rtb_d[si]) if t == "b" else (rte[si], rte_d[si])
                eng.dma_start(sb[:, :], dr[:, :]).then_inc(s_rtc[si], 16)
            for k in range(NG - 1):
                nsz = S_SLICES[k // 2][1]
                eng.wait_ge(s_mm, k + 1)
                nc.scalar.activation(
                    obs[k][:, :], pss[k % 8][:, :nsz], AF.Sigmoid
                ).then_inc(s_sig, 1)
            # final group: split sigmoid by columns; the first half's output
            # leaves on sync while scalar finishes + sends the second half
            n0, nsz = S_SLICES[NS - 1]
            h = nsz // 2
            eng.wait_ge(s_mm, NG)
            nc.scalar.activation(
                obs[NG - 1][:, :h], pss[(NG - 1) % 8][:, :h], AF.Sigmoid
            ).then_inc(s_sig, 1)
            nc.scalar.activation(
                obs[NG - 1][:, h:], pss[(NG - 1) % 8][:, h:nsz], AF.Sigmoid
            ).then_inc(s_sig, 1)
            eng.dma_start(
                out_d[128:256, n0 + h : n0 + nsz], obs[NG - 1][:, h:]
            ).then_inc(s_out, 16)

        @blk.gpsimd
        def _(eng):
            # m0 outputs (even groups) finish 1st, 3rd, ...
            for si, (n0, nsz) in enumerate(S_SLICES):
                eng.wait_ge(s_sig, 2 * si + 1)
                eng.dma_start(
                    out_d[0:128, n0 : n0 + nsz], obs[2 * si][:, :]
                ).then_inc(s_out, 16)

        @blk.tensor
        def _(eng):
            # warm-up: keep the PE busy on junk so its p-state ramps to full
            # clock while the first input slices stream in
            for _ in range(NWARM):
                nc.tensor.matmul(
                    pss[7][:, :], dummy[:, 0:128], dummy[:, :],
                    start=True, stop=True,
                )
            lhe3 = lhe[0:100, :].rearrange("p (two f) -> p two f", two=2)
            lh0 = lhb[:, 0 : 2 * B].bitcast(BF16)       # k0/k1 rows, bf16
            lh1 = lhb[:, 2 * B : 3 * B].bitcast(F8)     # k2/k3 rows, fp8
            for si, (n0, nsz) in enumerate(S_SLICES):
                eng.wait_ge(s_rtc[si], 32)
                if si == 0:
                    eng.wait_ge(s_lh, 32)
                t0b = rtb[si][:, 0 : 2 * nsz].bitcast(BF16)
                t1f = rtb[si][:, 2 * nsz : 3 * nsz].bitcast(F8)
                rte3 = rte[si][0:100, :].rearrange("p (two f) -> p two f", two=2)
                for m in range(2):
                    k = 2 * si + m
                    ps = pss[k % 8]
                    if k >= 8:
                        # bank recycled from group k-8: its sigmoid must be done
                        eng.wait_ge(s_sig, k - 7)
                    nc.tensor.matmul(
                        ps[:, :nsz],
                        lh0[:, m * 128 : (m + 1) * 128],
                        t0b,
                        start=True,
                        stop=False,
                    )
                    nc.tensor.matmul(
                        ps[:, :nsz],
                        lh1[:, m * 128 : (m + 1) * 128],
                        t1f,
                        start=False,
                        stop=False,
                    )
                    nc.tensor.matmul(
                        ps[:, :nsz],
                        lhe3[:, :, m * 128 : (m + 1) * 128],
                        rte3,
                        start=False,
                        stop=True,
                        perf_mode=DR,
                    ).then_inc(s_mm, 1)

    nc.compile()
    return nc


def _host_prep(emb_e, emb_rel, nf_weights, lit, c, var, e1, rel):
    import ml_dtypes

    bf = ml_dtypes.bfloat16
    f8 = ml_dtypes.float8_e4m3
    e1 = np.asarray(e1).astype(np.int64)
    rel = np.asarray(rel).astype(np.int64)
    lit64 = np.asarray(lit, np.float64)
    c64 = np.asarray(c, np.float64)
    var64 = np.asarray(var, np.float64)

    rsv = 1.0 / np.sqrt(var64)                      # (L,)
    g = -c64 * rsv

    # ---- lhs side (batch): A[b, k*64+l] (bf16) and emb rows (fp8)
    P = lit64[e1]                                   # (B, L)
    w = np.asarray(nf_weights, np.float64)[rel]     # (B, L)
    amg = (P - 0.5) * rsv                           # alpha - g
    alpha = amg + g
    u = np.exp(-(alpha**2)) * w                     # (B, L)
    t2 = 2.0 * amg
    lhsT = np.zeros((KT * L, B), np.float64)
    acc = u.copy()
    for k in range(KT):
        if k:
            acc = acc * t2
        lhsT[k * L : (k + 1) * L, :] = (CHEB[k] * acc).T
    lhb = np.zeros((128, 3 * B), np.uint8)
    lhb[:, : 2 * B] = (
        lhsT[:128].astype(bf).view(np.uint8).reshape(128, 2 * B)
    )
    lhb[:, 2 * B :] = lhsT[128:].astype(f8).view(np.uint8).reshape(128, B)
    x = np.asarray(emb_e, np.float64)[e1] * np.asarray(emb_rel, np.float64)[rel]
    lhe = np.ascontiguousarray(
        x.T.astype(f8).reshape(2, 100, B).transpose(1, 0, 2).reshape(100, 2 * B)
    )

    # ---- rhs side (entities): Bt[k*64+l, e] = V * beta^k (bf16), emb_e.T (fp8)
    beta = (lit64 - 0.5) * rsv                      # (E, L)
    V = np.exp(beta * (2.0 * g - beta))             # (E, L)
    rhs = np.empty((KT * L, E), np.float64)
    accr = V.copy()
    for k in range(KT):
        if k:
            accr = accr * beta
        rhs[k * L : (k + 1) * L, :] = accr.T
    eT8 = np.asarray(emb_e, np.float64).T.astype(f8)   # (D, E)

    in_maps = []
    for ci in range(NCORES):
        lo = ci * ES
        R0 = rhs[:128, lo : lo + ES]
        R1 = rhs[128:, lo : lo + ES]
        Ej = eT8[:, lo : lo + ES].reshape(2, 100, ES)
        m = {"lhb": lhb, "lhe": lhe}
        for si, (n0, nsz) in enumerate(S_SLICES):
            t = np.zeros((128, 3 * nsz + (nsz % 2)), np.uint8)
            t[:, : 2 * nsz] = (
                R0[:, n0 : n0 + nsz].astype(bf).view(np.uint8)
            )
            t[:, 2 * nsz : 3 * nsz] = R1[:, n0 : n0 + nsz].astype(f8).view(np.uint8)
            m[f"rtb{si}"] = t
            m[f"rte{si}"] = np.ascontiguousarray(
                Ej[:, :, n0 : n0 + nsz].transpose(1, 0, 2).reshape(100, 2 * nsz)
            )
        in_maps.append(m)
    return in_maps


def kernel(emb_e, emb_rel, nf_weights, lit, c, var, e1, rel):
    global _PROG, LAST
    if _PROG is None:
        _PROG = _build_program()
    in_maps = _host_prep(emb_e, emb_rel, nf_weights, lit, c, var, e1, rel)
    res = bass_utils.run_bass_kernel_spmd(
        _PROG, in_maps, core_ids=list(range(NCORES)), trace=TRACE
    )
    LAST = res
    return np.concatenate(
        [np.asarray(res.results[ci]["out"]).astype(np.float32) for ci in range(NCORES)],
        axis=1,
    )
